# revision 26
# baseline (speedup 1.0000x reference)
"""Trainium2 Bass kernel for nn_MatcherDynamicK (DynamicK OTA matcher).

Strategy (8 NeuronCores, data-parallel over B=16 -> 2 images/core):
  Device (per image, M-major layout: gt on partitions, anchors on free dim):
    - focal-cost prelude on [N, C] logits (ACT sigmoid/ln/square, GPSIMD mults)
    - cost_class gather as PE matmul with one-hot(labels) lhsT (K=81, const
      row folds the "-4" constant)
    - pairwise IoU / enclose / L1-bbox grids via custom fused DVE ops
      (WH / EWH / BBOX2 / TTS) + GPSIMD tensor ops + fast reciprocal
    - emits negcost^T [M, N] (negated cost) and per-chunk top-8 iou candidates
  Host: top-10-cost per gt from the negcost matrix, dynamic-k selection,
    conflict resolution (argmin rows/cols), builds sel / gt_indices / cost.
"""
import numpy as np

import concourse.bacc as bacc
import concourse.mybir as mybir
import concourse.tile as tile
from concourse.bass_utils import run_bass_kernel_spmd
from concourse.masks import make_identity

import concourse.dve_ops as dve_ops
from concourse.dve_spec import (
    Spec, Src0, Src1, C0, C1, lower, minn, maxx, relu, _has_src1,
)
from concourse.dve_uop import DveOpSpec

# ---------------- problem constants (fixed by the graded problem) ----------
B, N, C, M = 16, 4096, 80, 256
OTA_K = 10
P = 128
CHUNK = 512
NCH = N // CHUNK          # 8
IPC = 2                   # images per core
NCORES = 8
NCAND = NCH * 8           # 64 iou candidates per gt row
KROWS = 5                 # replicated anchor rows: px1 py1 px2 py2 ap
F32 = mybir.dt.float32
U32 = mybir.dt.uint32

# ---------------- custom DVE op registration -------------------------------


def _register_op(name, body, ref):
    if name in dve_ops._SUB_OPCODE_FOR_NAME:   # idempotent on re-import
        for op in dve_ops.OPS:
            if op.name == name:
                return op
    spec = Spec(body=body, reference=ref)
    shas = {}
    op = dve_ops.DveOp(name, spec, subdim=False, uops_sha=shas)
    dve_ops.OPS.append(op)
    dve_ops.CUSTOM_DVE_SPECS[name] = spec
    dve_ops._SUB_OPCODE_FOR_NAME[name] = (
        dve_ops._CUSTOM_DVE_ROW_BASE + len(dve_ops.OPS) - 1
    )
    assert dve_ops._SUB_OPCODE_FOR_NAME[name] < 0x20
    for ver in ("v3", "v4"):
        s = DveOpSpec(name=name, opcode=dve_ops.get_dve_sub_opcode(name),
                      uops=lower(spec, ver=ver), rd1_en=_has_src1(spec))
        shas[ver] = s.sha(ver)
    return op


OP_WH = _register_op(
    "ANT_WH",
    relu(minn(Src0, C0) - maxx(Src1, C1)),
    lambda in0, in1, c0, c1, c2: np.maximum(
        np.minimum(in0, c0) - np.maximum(in1, c1), 0).astype(np.float32),
)
OP_EWH = _register_op(
    "ANT_EWH",
    maxx(Src0, C0) - minn(Src1, C1),
    lambda in0, in1, c0, c1, c2: (
        np.maximum(in0, c0) - np.minimum(in1, c1)).astype(np.float32),
)
OP_BBOX2 = _register_op(
    "ANT_BBOX2",
    maxx(Src0 - C0, C0 - Src0) + maxx(Src1 - C1, C1 - Src1),
    lambda in0, in1, c0, c1, c2: (
        np.abs(in0 - c0) + np.abs(in1 - c1)).astype(np.float32),
)
OP_TTS = _register_op(
    "ANT_TTS",
    Src0 * Src1 * C0,
    lambda in0, in1, c0, c1, c2: (in0 * in1 * c0).astype(np.float32),
)

# ---------------- device program build -------------------------------------

_NC_CACHE = None


def _build_nc(opts=None):
    opts = dict(opts or {})
    ue_act = opts.get("ue_act", False)      # enclose branch via ACT exp-ln
    sxy_act = opts.get("sxy_act", False)    # bbox scaling on ACT
    union_dve = opts.get("union_dve", False)
    bby_act = opts.get("bby_act", False)    # bby via ACT Abs x2 + Pool add
    bbx_act = opts.get("bbx_act", False)
    rows_bufs = opts.get("rows_bufs", 2)
    grid_bufs = opts.get("grid_bufs", 2)
    ps_bufs = opts.get("ps_bufs", 2)
    nc = bacc.Bacc(None)
    AF = mybir.ActivationFunctionType

    lg = nc.dram_tensor("lg", [IPC, P, 32 * C], F32, kind="ExternalInput")
    # prow rows: px1 py1 px2 py2 ap  const(-4)
    prow = nc.dram_tensor("prow", [IPC, KROWS + 1, N], F32, kind="ExternalInput")
    # gcol cols: gx1 gy1 gx2 gy2 ag cbx1 cby1 cbx2 cby2 sx sy
    gcol = nc.dram_tensor("gcol", [IPC, 2, P, 15], F32, kind="ExternalInput")
    oh = nc.dram_tensor("oh", [IPC, 81, M], F32, kind="ExternalInput")
    # selection lhsT for row replication: emat[k, r*P+p] = (k == r)
    emat = nc.dram_tensor("emat", [KROWS, KROWS * P], F32, kind="ExternalInput")

    negc = nc.dram_tensor("negc", [IPC, 2, P, N], F32, kind="ExternalOutput")
    icv = nc.dram_tensor("icv", [IPC, 2, P, NCAND], F32, kind="ExternalOutput")

    with tile.TileContext(nc) as tc:
        with (
            tc.tile_pool(name="const", bufs=1) as cst,
            tc.tile_pool(name="pre", bufs=1) as pre,
            tc.tile_pool(name="gt_pool", bufs=1) as gtp,
            tc.tile_pool(name="rows", bufs=rows_bufs) as rows,
            tc.tile_pool(name="grid", bufs=grid_bufs) as grid,
            tc.tile_pool(name="cand", bufs=2) as cand,
            tc.tile_pool(name="ps", bufs=ps_bufs, space="PSUM") as ps,
        ):
            ident = cst.tile([P, P], F32)
            make_identity(nc, ident)
            sb_e = cst.tile([KROWS, KROWS * P], F32)
            nc.gpsimd.dma_start(sb_e[:], emat[:])
            b_zero = cst.tile([P, 1], F32)
            nc.gpsimd.memset(b_zero[:], 0.0)
            b_one = cst.tile([P, 1], F32)
            nc.gpsimd.memset(b_one[:], 1.0)
            b_lq = cst.tile([P, 1], F32)
            nc.gpsimd.memset(b_lq[:], -1.3862943611198906)   # ln(1/4)
            b_l3q = cst.tile([P, 1], F32)
            nc.gpsimd.memset(b_l3q[:], -0.2876820724517809)  # ln(3/4)
            b_ln2 = cst.tile([P, 1], F32)
            nc.gpsimd.memset(b_ln2[:], 0.6931471805599453)

            for i in range(IPC):
                # grid-side inputs first: the DVE grid pipeline depends only
                # on these, so their DMAs must not queue behind the big
                # logits transfer.
                sb_prow = gtp.tile([KROWS, N], F32, name=f"prow_{i}")
                nc.sync.dma_start(sb_prow[:], prow[i, 0:KROWS, :])
                sb_oh = gtp.tile([81, M], F32, name=f"oh_{i}")
                nc.gpsimd.dma_start(sb_oh[:], oh[i][:])
                gs = [gtp.tile([P, 15], F32, name=f"gs_{i}_{mt}") for mt in range(2)]
                for mt in range(2):
                    nc.gpsimd.dma_start(gs[mt][:], gcol[i, mt][:])

                # exp/ln-form focal prelude (single ACT table set):
                #   t=e^-x, L1b=ln(1+t)=-ln p ; s=e^x, L2b=ln(1+s)=-ln(1-p)
                #   u'=0.25(1-p)^2=exp(-2*L2b+ln1/4) ; v'=0.75p^2=exp(-2*L1b+ln3/4)
                #   gneg = v'*L2b - u'*L1b  (= -cost_class)
                sb_lg = pre.tile([P, 32 * C], F32, tag="pa", name=f"lg_{i}")
                nc.sync.dma_start(sb_lg[:], lg[i][:])
                sb_t = pre.tile([P, 32 * C], F32, tag="pb", name=f"t_{i}")
                nc.scalar.activation(sb_t[:], sb_lg[:], AF.Exp, bias=b_zero[:],
                                     scale=-1.0)
                sb_L1 = pre.tile([P, 32 * C], F32, tag="pc", name=f"L1_{i}")
                nc.scalar.activation(sb_L1[:], sb_t[:], AF.Ln, bias=b_one[:])
                sb_s = pre.tile([P, 32 * C], F32, tag="pb", name=f"s_{i}")
                nc.scalar.activation(sb_s[:], sb_lg[:], AF.Exp, bias=b_zero[:])
                sb_L2 = pre.tile([P, 32 * C], F32, tag="pd", name=f"L2_{i}")
                nc.scalar.activation(sb_L2[:], sb_s[:], AF.Ln, bias=b_one[:])
                sb_u = pre.tile([P, 32 * C], F32, tag="pe", name=f"u_{i}")
                nc.scalar.activation(sb_u[:], sb_L2[:], AF.Exp, bias=b_lq[:],
                                     scale=-2.0)
                sb_A = pre.tile([P, 32 * C], F32, tag="pa", name=f"A_{i}")
                nc.gpsimd.tensor_mul(sb_A[:], sb_u[:], sb_L1[:])
                sb_v = pre.tile([P, 32 * C], F32, tag="pe", name=f"v_{i}")
                nc.scalar.activation(sb_v[:], sb_L1[:], AF.Exp, bias=b_l3q[:],
                                     scale=-2.0)
                sb_Bv = pre.tile([P, 32 * C], F32, tag="pc", name=f"Bv_{i}")
                nc.gpsimd.tensor_mul(sb_Bv[:], sb_v[:], sb_L2[:])
                sb_gneg = pre.tile([P, 32 * C], F32, tag="pb", name=f"gneg_{i}")
                nc.gpsimd.tensor_sub(sb_gneg[:], sb_Bv[:], sb_A[:])

                # transpose to gnegT [81, N] (row 80 = -4 const via DMA)
                gnegT = gtp.tile([81, N], F32)
                nc.sync.dma_start(gnegT[80:81, :], prow[i, KROWS:KROWS + 1, :])
                for q in range(NCH):       # 512-wide PSUM stripes
                    tp_ps = ps.tile([C, CHUNK], F32, tag="ps_tp")
                    for j in range(CHUNK // P):   # 4 transposes of [128, 80]
                        t = q * (CHUNK // P) + j
                        nc.tensor.transpose(
                            tp_ps[:, j * P:(j + 1) * P],
                            sb_gneg[:, t * C:(t + 1) * C],
                            ident[:],
                        )
                    nc.scalar.copy(gnegT[0:C, q * CHUNK:(q + 1) * CHUNK], tp_ps[:])

                # iou candidate accumulators per m-tile
                t_icv = [cand.tile([P, NCAND], F32, name=f"icv_{i}_{mt}")
                         for mt in range(2)]
                # negcost output buffers [128, N] per m-tile (batched DMA)
                t_neg = [cand.tile([P, N], F32, tag=f"negbuf{mt}", bufs=1,
                                   name=f"neg_{i}_{mt}") for mt in range(2)]

                for ch in range(NCH):
                    sl = slice(ch * CHUNK, (ch + 1) * CHUNK)
                    # replicate anchor rows: selection matmul rhs=[5,512]
                    rr = []
                    for r in range(KROWS):
                        ps_r = ps.tile([P, CHUNK], F32, tag="ps_rep")
                        nc.tensor.matmul(ps_r[:], sb_e[:, r * P:(r + 1) * P],
                                         sb_prow[:, sl], start=True, stop=True)
                        sb_r = rows.tile([P, CHUNK], F32, tag=f"row{r}",
                                         name=f"rowR{r}_{i}_{ch}")
                        if i == 0 and ch == 0:
                            nc.vector.tensor_copy(sb_r[:], ps_r[:])
                        else:
                            nc.scalar.copy(sb_r[:], ps_r[:])
                        rr.append(sb_r)
                    r_px1, r_py1, r_px2, r_py2, r_ap = rr

                    for mt in range(2):
                        g = gs[mt]
                        g_x1, g_y1 = g[:, 0:1], g[:, 1:2]
                        g_x2, g_y2 = g[:, 2:3], g[:, 3:4]
                        g_ag = g[:, 4:5]
                        g_bx1, g_by1 = g[:, 5:6], g[:, 6:7]
                        g_bx2, g_by2 = g[:, 7:8], g[:, 8:9]
                        g_sx, g_sy = g[:, 9:10], g[:, 10:11]
                        g_nby1, g_nby2 = g[:, 11:12], g[:, 12:13]
                        g_nbx1, g_nbx2 = g[:, 13:14], g[:, 14:15]

                        def gt_tile(nm, dt=F32, tg=None):
                            return grid.tile([P, CHUNK], dt, tag=tg or nm,
                                             name=f"{nm}_{i}_{ch}_{mt}")

                        # cost_class psum -> SBUF (ACT copy)
                        ps_cc = ps.tile([P, CHUNK], F32, tag="ps_cc",
                                        name=f"pscc_{i}_{ch}_{mt}")
                        nc.tensor.matmul(ps_cc[:], sb_oh[:, mt * P:(mt + 1) * P],
                                         gnegT[:, sl], start=True, stop=True)
                        cc_sb = gt_tile("cc_sb")
                        nc.scalar.copy(cc_sb[:], ps_cc[:])

                        wx = gt_tile("wx")
                        nc.vector._custom_dve(OP_WH, out=wx[:], in0=r_px2[:],
                                              in1=r_px1[:], s0=g_x2, s1=g_x1,
                                              imm2=0.0)
                        wy = gt_tile("wy")
                        nc.vector._custom_dve(OP_WH, out=wy[:], in0=r_py2[:],
                                              in1=r_py1[:], s0=g_y2, s1=g_y1,
                                              imm2=0.0)
                        inter = gt_tile("inter")
                        nc.gpsimd.tensor_mul(inter[:], wx[:], wy[:])
                        ewx = gt_tile("ewx", tg="wx")
                        nc.vector._custom_dve(OP_EWH, out=ewx[:], in0=r_px2[:],
                                              in1=r_px1[:], s0=g_x2, s1=g_x1,
                                              imm2=0.0)
                        ewy = gt_tile("ewy", tg="wy")
                        nc.vector._custom_dve(OP_EWH, out=ewy[:], in0=r_py2[:],
                                              in1=r_py1[:], s0=g_y2, s1=g_y1,
                                              imm2=0.0)
                        encl = gt_tile("encl")
                        nc.gpsimd.tensor_mul(encl[:], ewx[:], ewy[:])
                        union = gt_tile("union")
                        if union_dve:
                            nc.vector.scalar_tensor_tensor(
                                union[:], r_ap[:], g_ag, inter[:],
                                mybir.AluOpType.add, mybir.AluOpType.subtract)
                        else:
                            nc.gpsimd.tensor_scalar_add(union[:], r_ap[:], g_ag)
                            nc.gpsimd.tensor_sub(union[:], union[:], inter[:])
                        r_u = gt_tile("r_u")
                        nc.vector.reciprocal_approx_fast(out=r_u[:], in_=union[:])
                        iou2 = gt_tile("iou2")
                        nc.vector._custom_dve(OP_TTS, out=iou2[:], in0=inter[:],
                                              in1=r_u[:], s0=2.0, s1=0.0, imm2=0.0)
                        ue2 = gt_tile("ue2")
                        if ue_act:
                            lnu = gt_tile("lnu", tg="r_e")
                            nc.scalar.activation(lnu[:], union[:], AF.Ln,
                                                 bias=b_zero[:])
                            lne = gt_tile("lne", tg="wy")
                            nc.scalar.activation(lne[:], encl[:], AF.Ln,
                                                 bias=b_zero[:])
                            dl = gt_tile("dl", tg="union")
                            nc.gpsimd.tensor_sub(dl[:], lnu[:], lne[:])
                            nc.scalar.activation(ue2[:], dl[:], AF.Exp,
                                                 bias=b_ln2[:])
                        else:
                            r_e = gt_tile("r_e")
                            nc.vector.reciprocal_approx_fast(out=r_e[:], in_=encl[:])
                            nc.vector._custom_dve(OP_TTS, out=ue2[:], in0=union[:],
                                                  in1=r_e[:], s0=2.0, s1=0.0,
                                                  imm2=0.0)
                        bbx = gt_tile("bbx")
                        if bbx_act:
                            ax1 = gt_tile("ax1", tg="inter2" if False else "encl")
                            nc.scalar.activation(ax1[:], r_px1[:], AF.Abs,
                                                 bias=g_nbx1)
                            ax2 = gt_tile("ax2", tg="union")
                            nc.scalar.activation(ax2[:], r_px2[:], AF.Abs,
                                                 bias=g_nbx2)
                            nc.gpsimd.tensor_add(bbx[:], ax1[:], ax2[:])
                        else:
                            nc.vector._custom_dve(OP_BBOX2, out=bbx[:],
                                                  in0=r_px1[:], in1=r_px2[:],
                                                  s0=g_bx1, s1=g_bx2, imm2=0.0)
                        bby = gt_tile("bby")
                        if bby_act:
                            ab1 = gt_tile("ab1", tg="wx")
                            nc.scalar.activation(ab1[:], r_py1[:], AF.Abs,
                                                 bias=g_nby1, scale=g_sy)
                            ab2 = gt_tile("ab2", tg="wy")
                            nc.scalar.activation(ab2[:], r_py2[:], AF.Abs,
                                                 bias=g_nby2, scale=g_sy)
                            nc.gpsimd.tensor_add(bby[:], ab1[:], ab2[:])
                        else:
                            nc.vector._custom_dve(OP_BBOX2, out=bby[:],
                                                  in0=r_py1[:], in1=r_py2[:],
                                                  s0=g_by1, s1=g_by2, imm2=0.0)
                        # merges on GPSIMD (in-place where safe)
                        nc.gpsimd.tensor_add(ue2[:], ue2[:], iou2[:])      # m1
                        if sxy_act:
                            nc.scalar.activation(bbx[:], bbx[:], AF.Copy,
                                                 scale=g_sx)
                        else:
                            nc.gpsimd.tensor_scalar_mul(bbx[:], bbx[:], g_sx)
                        if not bby_act:
                            nc.gpsimd.tensor_scalar_mul(bby[:], bby[:], g_sy)
                        nc.gpsimd.tensor_add(bbx[:], bbx[:], bby[:])       # m2
                        nc.gpsimd.tensor_sub(ue2[:], ue2[:], bbx[:])       # m3
                        nc.gpsimd.tensor_add(t_neg[mt][:, sl], ue2[:], cc_sb[:])

                        csl = slice(ch * 8, (ch + 1) * 8)
                        nc.vector.max(out=t_icv[mt][:, csl], in_=iou2[:])

                for mt in range(2):
                    nc.scalar.dma_start(negc[i, mt, :, 0:N // 2],
                                        t_neg[mt][:, 0:N // 2])
                    nc.sync.dma_start(negc[i, mt, :, N // 2:N],
                                      t_neg[mt][:, N // 2:N])
                    nc.gpsimd.dma_start(icv[i, mt][:], t_icv[mt][:])

    nc.compile()
    return nc


BEST_OPTS = {"bby_act": True}


def _get_nc():
    global _NC_CACHE
    if _NC_CACHE is None:
        _NC_CACHE = _build_nc(BEST_OPTS)
    return _NC_CACHE


# ---------------- host side ------------------------------------------------


def _pack_inputs(pred_logits, pred_boxes, boxes_xyxy, labels,
                 image_size_xyxy, image_size_xyxy_tgt):
    f32 = np.float32
    pred_logits = np.asarray(pred_logits, f32)
    pred_boxes = np.asarray(pred_boxes, f32)
    boxes_xyxy = np.asarray(boxes_xyxy, f32)
    labels = np.asarray(labels)
    img = np.asarray(image_size_xyxy, f32)
    img_t = np.asarray(image_size_xyxy_tgt, f32)

    # lg: [B, 128, 32*C] packed so SBUF partition p holds anchors {t*128+p}
    lg = np.ascontiguousarray(
        pred_logits.reshape(B, 32, P, C).transpose(0, 2, 1, 3)
    ).reshape(B, P, 32 * C)

    px1, py1, px2, py2 = [pred_boxes[:, :, j] for j in range(4)]
    ap = (px2 - px1) * (py2 - py1)
    const4 = np.full((B, N), -4.0, f32)
    prow = np.stack([px1, py1, px2, py2, ap, const4], axis=1)

    gx1, gy1, gx2, gy2 = [boxes_xyxy[:, :, j] for j in range(4)]
    ag = (gx2 - gx1) * (gy2 - gy1)
    # bbox scaling: 5*|px/w - gx/wt| = (5/w)*|px - gx*(w/wt)|
    w, h = img[:, 0:1], img[:, 1:2]
    wt, ht = img_t[:, 0:1], img_t[:, 1:2]
    sx = np.broadcast_to(5.0 / w, (B, M)).astype(f32)
    sy = np.broadcast_to(5.0 / h, (B, M)).astype(f32)
    gcol = np.stack([gx1, gy1, gx2, gy2, ag,
                     gx1 * (w / wt), gy1 * (h / ht),
                     gx2 * (w / wt), gy2 * (h / ht), sx, sy,
                     -5.0 * gy1 / ht, -5.0 * gy2 / ht,
                     -gx1 * (w / wt), -gx2 * (w / wt)],
                    axis=2).astype(f32)          # [B, M, 15]
    gcol = gcol.reshape(B, 2, P, 15)

    ohot = np.zeros((B, 81, M), f32)
    ohot[:, 80, :] = 1.0
    bidx = np.repeat(np.arange(B), M)
    ohot[bidx, labels.astype(np.int64).ravel(), np.tile(np.arange(M), B)] = 1.0

    emat = np.zeros((KROWS, KROWS * P), f32)
    for r in range(KROWS):
        emat[r, r * P:(r + 1) * P] = 1.0

    in_maps = []
    for c in range(NCORES):
        s = slice(IPC * c, IPC * (c + 1))
        in_maps.append({
            "lg": np.ascontiguousarray(lg[s]),
            "prow": np.ascontiguousarray(prow[s].astype(f32)),
            "gcol": np.ascontiguousarray(gcol[s]),
            "oh": np.ascontiguousarray(ohot[s]),
            "emat": emat,
        })
    return in_maps


def _assemble(negcT, icv):
    """negcT [B, M, N] f32 (= -cost^T); icv [B, M, NCAND] f32 (~2*iou)."""
    f32 = np.float32
    R = B * M

    # dyn_k from iou candidates (values are 2*iou)
    iv = -np.sort(-icv.reshape(R, NCAND), axis=1)[:, :OTA_K] * f32(0.5)
    dyn_k = np.maximum(iv.sum(1).astype(np.int32), 1)            # [R]

    # exact top-10 smallest cost (= largest negcost), ties -> lowest index
    flat = negcT.reshape(R, N)
    part = np.argpartition(-flat, OTA_K + 2, axis=1)[:, :OTA_K + 2]
    pv = np.take_along_axis(flat, part, 1)
    order = np.lexsort((part, -pv), axis=-1)[:, :OTA_K]
    idx10 = np.take_along_axis(part, order, 1)                   # [R, 10]

    keep = np.arange(OTA_K)[None, :] < dyn_k[:, None]            # [R, 10]
    mm = np.zeros((B, N, M), bool)
    bb = np.broadcast_to(np.repeat(np.arange(B), M)[:, None], (R, OTA_K))
    mcol = np.broadcast_to(np.tile(np.arange(M), B)[:, None], (R, OTA_K))
    mm[bb[keep], idx10[keep], mcol[keep]] = True

    s = mm.sum(2)
    multi = s > 1
    rm = np.argmax(negcT, axis=1)            # [B, N] argmin-cost gt per anchor
    bbn, nn = np.nonzero(multi)
    mm[bbn, nn, :] = False
    mm[bbn, nn, rm[bbn, nn]] = True

    u = mm.sum(1)
    un = u == 0                              # [B, M]
    cm = np.argmax(negcT, axis=2)            # [B, M] argmin-cost anchor per gt
    bbm, mmi = np.nonzero(un)
    mm[bbm, cm[bbm, mmi], mmi] = True

    sel = mm.any(2)
    gt_idx = np.argmax(mm, axis=2).astype(np.int32)
    cost = (-negcT).transpose(0, 2, 1)
    return sel, gt_idx, cost


def kernel(pred_logits, pred_boxes, boxes_xyxy, labels,
           image_size_xyxy, image_size_xyxy_tgt):
    nc = _get_nc()
    in_maps = _pack_inputs(pred_logits, pred_boxes, boxes_xyxy, labels,
                           image_size_xyxy, image_size_xyxy_tgt)
    res = run_bass_kernel_spmd(nc, in_maps, list(range(NCORES)))
    outs = res.results

    negcT = np.empty((B, M, N), np.float32)
    icv = np.empty((B, M, NCAND), np.float32)
    for c in range(NCORES):
        o = outs[c]
        for i in range(IPC):
            b = IPC * c + i
            negcT[b] = o["negc"][i].reshape(M, N)
            icv[b] = o["icv"][i].reshape(M, NCAND)

    return _assemble(negcT, icv)


# revision 32
# speedup vs baseline: 1.0068x; 1.0068x over previous
"""Trainium2 Bass kernel for nn_MatcherDynamicK (DynamicK OTA matcher).

Strategy (8 NeuronCores, data-parallel over B=16 -> 2 images/core):
  Device (per image, M-major layout: gt on partitions, anchors on free dim):
    - focal-cost prelude on [N, C] logits (ACT sigmoid/ln/square, GPSIMD mults)
    - cost_class gather as PE matmul with one-hot(labels) lhsT (K=81, const
      row folds the "-4" constant)
    - pairwise IoU / enclose / L1-bbox grids via custom fused DVE ops
      (WH / EWH / BBOX2 / TTS) + GPSIMD tensor ops + fast reciprocal
    - emits negcost^T [M, N] (negated cost) and per-chunk top-8 iou candidates
  Host: top-10-cost per gt from the negcost matrix, dynamic-k selection,
    conflict resolution (argmin rows/cols), builds sel / gt_indices / cost.
"""
import numpy as np

import concourse.bacc as bacc
import concourse.mybir as mybir
import concourse.tile as tile
from concourse.bass_utils import run_bass_kernel_spmd
from concourse.masks import make_identity

import concourse.dve_ops as dve_ops
from concourse.dve_spec import (
    Spec, Src0, Src1, C0, C1, lower, minn, maxx, relu, _has_src1,
)
from concourse.dve_uop import DveOpSpec

# ---------------- problem constants (fixed by the graded problem) ----------
B, N, C, M = 16, 4096, 80, 256
OTA_K = 10
P = 128
CHUNK = 512
NCH = N // CHUNK          # 8
IPC = 2                   # images per core
NCORES = 8
NCAND = NCH * 8           # 64 iou candidates per gt row
KROWS = 5                 # replicated anchor rows: px1 py1 px2 py2 ap
F32 = mybir.dt.float32
U32 = mybir.dt.uint32

# ---------------- custom DVE op registration -------------------------------


def _register_op(name, body, ref):
    if name in dve_ops._SUB_OPCODE_FOR_NAME:   # idempotent on re-import
        for op in dve_ops.OPS:
            if op.name == name:
                return op
    spec = Spec(body=body, reference=ref)
    shas = {}
    op = dve_ops.DveOp(name, spec, subdim=False, uops_sha=shas)
    dve_ops.OPS.append(op)
    dve_ops.CUSTOM_DVE_SPECS[name] = spec
    dve_ops._SUB_OPCODE_FOR_NAME[name] = (
        dve_ops._CUSTOM_DVE_ROW_BASE + len(dve_ops.OPS) - 1
    )
    assert dve_ops._SUB_OPCODE_FOR_NAME[name] < 0x20
    for ver in ("v3", "v4"):
        s = DveOpSpec(name=name, opcode=dve_ops.get_dve_sub_opcode(name),
                      uops=lower(spec, ver=ver), rd1_en=_has_src1(spec))
        shas[ver] = s.sha(ver)
    return op


OP_WH = _register_op(
    "ANT_WH",
    relu(minn(Src0, C0) - maxx(Src1, C1)),
    lambda in0, in1, c0, c1, c2: np.maximum(
        np.minimum(in0, c0) - np.maximum(in1, c1), 0).astype(np.float32),
)
OP_EWH = _register_op(
    "ANT_EWH",
    maxx(Src0, C0) - minn(Src1, C1),
    lambda in0, in1, c0, c1, c2: (
        np.maximum(in0, c0) - np.minimum(in1, c1)).astype(np.float32),
)
OP_BBOX2 = _register_op(
    "ANT_BBOX2",
    maxx(Src0 - C0, C0 - Src0) + maxx(Src1 - C1, C1 - Src1),
    lambda in0, in1, c0, c1, c2: (
        np.abs(in0 - c0) + np.abs(in1 - c1)).astype(np.float32),
)
OP_TTS = _register_op(
    "ANT_TTS",
    Src0 * Src1 * C0,
    lambda in0, in1, c0, c1, c2: (in0 * in1 * c0).astype(np.float32),
)

# ---------------- device program build -------------------------------------

_NC_CACHE = None


def _build_nc(opts=None):
    opts = dict(opts or {})
    ue_act = opts.get("ue_act", False)      # enclose branch via ACT exp-ln
    sxy_act = opts.get("sxy_act", False)    # bbox scaling on ACT
    union_dve = opts.get("union_dve", False)
    bby_act = opts.get("bby_act", False)    # bby via ACT Abs x2 + Pool add
    bbx_act = opts.get("bbx_act", False)
    rows_bufs = opts.get("rows_bufs", 2)
    grid_bufs = opts.get("grid_bufs", 2)
    ps_bufs = opts.get("ps_bufs", 2)
    nc = bacc.Bacc(None)
    AF = mybir.ActivationFunctionType

    lg = nc.dram_tensor("lg", [IPC, P, 32 * C], F32, kind="ExternalInput")
    # prow rows: px1 py1 px2 py2 ap  const(-4)
    prow = nc.dram_tensor("prow", [IPC, KROWS + 1, N], F32, kind="ExternalInput")
    # gcol cols: gx1 gy1 gx2 gy2 ag cbx1 cby1 cbx2 cby2 sx sy
    gcol = nc.dram_tensor("gcol", [IPC, 2, P, 15], F32, kind="ExternalInput")
    oh = nc.dram_tensor("oh", [IPC, 81, M], F32, kind="ExternalInput")
    # selection lhsT for row replication: emat[k, r*P+p] = (k == r)
    emat = nc.dram_tensor("emat", [KROWS, KROWS * P], F32, kind="ExternalInput")

    negc = nc.dram_tensor("negc", [IPC, 2, P, N], F32, kind="ExternalOutput")
    icv = nc.dram_tensor("icv", [IPC, 2, P, NCAND], F32, kind="ExternalOutput")

    with tile.TileContext(nc) as tc:
        with (
            tc.tile_pool(name="const", bufs=1) as cst,
            tc.tile_pool(name="pre", bufs=1) as pre,
            tc.tile_pool(name="gt_pool", bufs=1) as gtp,
            tc.tile_pool(name="rows", bufs=rows_bufs) as rows,
            tc.tile_pool(name="grid", bufs=grid_bufs) as grid,
            tc.tile_pool(name="cand", bufs=2) as cand,
            tc.tile_pool(name="ps", bufs=ps_bufs, space="PSUM") as ps,
        ):
            ident = cst.tile([P, P], F32)
            make_identity(nc, ident)
            sb_e = cst.tile([KROWS, KROWS * P], F32)
            nc.gpsimd.dma_start(sb_e[:], emat[:])
            b_zero = cst.tile([P, 1], F32)
            nc.gpsimd.memset(b_zero[:], 0.0)
            b_one = cst.tile([P, 1], F32)
            nc.gpsimd.memset(b_one[:], 1.0)
            b_lq = cst.tile([P, 1], F32)
            nc.gpsimd.memset(b_lq[:], -1.3862943611198906)   # ln(1/4)
            b_l3q = cst.tile([P, 1], F32)
            nc.gpsimd.memset(b_l3q[:], -0.2876820724517809)  # ln(3/4)
            b_ln2 = cst.tile([P, 1], F32)
            nc.gpsimd.memset(b_ln2[:], 0.6931471805599453)

            for i in range(IPC):
                # grid-side inputs first: the DVE grid pipeline depends only
                # on these, so their DMAs must not queue behind the big
                # logits transfer.
                sb_prow = gtp.tile([KROWS, N], F32, name=f"prow_{i}")
                nc.sync.dma_start(sb_prow[:], prow[i, 0:KROWS, :])
                sb_oh = gtp.tile([81, M], F32, name=f"oh_{i}")
                nc.gpsimd.dma_start(sb_oh[:], oh[i][:])
                gs = [gtp.tile([P, 15], F32, name=f"gs_{i}_{mt}") for mt in range(2)]
                for mt in range(2):
                    nc.gpsimd.dma_start(gs[mt][:], gcol[i, mt][:])

                # exp/ln-form focal prelude (single ACT table set):
                #   t=e^-x, L1b=ln(1+t)=-ln p ; s=e^x, L2b=ln(1+s)=-ln(1-p)
                #   u'=0.25(1-p)^2=exp(-2*L2b+ln1/4) ; v'=0.75p^2=exp(-2*L1b+ln3/4)
                #   gneg = v'*L2b - u'*L1b  (= -cost_class)
                sb_lg = pre.tile([P, 32 * C], F32, tag="pa", name=f"lg_{i}")
                nc.sync.dma_start(sb_lg[:], lg[i][:])
                sb_t = pre.tile([P, 32 * C], F32, tag="pb", name=f"t_{i}")
                nc.scalar.activation(sb_t[:], sb_lg[:], AF.Exp, bias=b_zero[:],
                                     scale=-1.0)
                sb_L1 = pre.tile([P, 32 * C], F32, tag="pc", name=f"L1_{i}")
                nc.scalar.activation(sb_L1[:], sb_t[:], AF.Ln, bias=b_one[:])
                sb_s = pre.tile([P, 32 * C], F32, tag="pb", name=f"s_{i}")
                nc.scalar.activation(sb_s[:], sb_lg[:], AF.Exp, bias=b_zero[:])
                sb_L2 = pre.tile([P, 32 * C], F32, tag="pd", name=f"L2_{i}")
                nc.scalar.activation(sb_L2[:], sb_s[:], AF.Ln, bias=b_one[:])
                sb_u = pre.tile([P, 32 * C], F32, tag="pe", name=f"u_{i}")
                nc.scalar.activation(sb_u[:], sb_L2[:], AF.Exp, bias=b_lq[:],
                                     scale=-2.0)
                sb_A = pre.tile([P, 32 * C], F32, tag="pa", name=f"A_{i}")
                nc.gpsimd.tensor_mul(sb_A[:], sb_u[:], sb_L1[:])
                sb_v = pre.tile([P, 32 * C], F32, tag="pe", name=f"v_{i}")
                nc.scalar.activation(sb_v[:], sb_L1[:], AF.Exp, bias=b_l3q[:],
                                     scale=-2.0)
                sb_Bv = pre.tile([P, 32 * C], F32, tag="pc", name=f"Bv_{i}")
                nc.gpsimd.tensor_mul(sb_Bv[:], sb_v[:], sb_L2[:])
                sb_gneg = pre.tile([P, 32 * C], F32, tag="pb", name=f"gneg_{i}")
                nc.gpsimd.tensor_sub(sb_gneg[:], sb_Bv[:], sb_A[:])

                # transpose to gnegT [81, N] (row 80 = -4 const via DMA)
                gnegT = gtp.tile([81, N], F32)
                nc.sync.dma_start(gnegT[80:81, :], prow[i, KROWS:KROWS + 1, :])
                for q in range(NCH):       # 512-wide PSUM stripes
                    tp_ps = ps.tile([C, CHUNK], F32, tag="ps_tp")
                    for j in range(CHUNK // P):   # 4 transposes of [128, 80]
                        t = q * (CHUNK // P) + j
                        nc.tensor.transpose(
                            tp_ps[:, j * P:(j + 1) * P],
                            sb_gneg[:, t * C:(t + 1) * C],
                            ident[:],
                        )
                    nc.scalar.copy(gnegT[0:C, q * CHUNK:(q + 1) * CHUNK], tp_ps[:])

                # iou candidate accumulators per m-tile
                t_icv = [cand.tile([P, NCAND], F32, bufs=1, name=f"icv_{i}_{mt}")
                         for mt in range(2)]
                # negcost output buffers [128, N] per m-tile (batched DMA)
                t_neg = [[cand.tile([P, N // 2], F32, tag=f"negbuf{mt}{hf}",
                                    bufs=1, name=f"neg_{i}_{mt}_{hf}")
                          for hf in range(2)] for mt in range(2)]

                for ch in range(NCH):
                    sl = slice(ch * CHUNK, (ch + 1) * CHUNK)
                    # replicate anchor rows: selection matmul rhs=[5,512]
                    rr = []
                    for r in range(KROWS):
                        ps_r = ps.tile([P, CHUNK], F32, tag="ps_rep")
                        nc.tensor.matmul(ps_r[:], sb_e[:, r * P:(r + 1) * P],
                                         sb_prow[:, sl], start=True, stop=True)
                        sb_r = rows.tile([P, CHUNK], F32, tag=f"row{r}",
                                         name=f"rowR{r}_{i}_{ch}")
                        if i == 0 and ch == 0:
                            nc.vector.tensor_copy(sb_r[:], ps_r[:])
                        else:
                            nc.scalar.copy(sb_r[:], ps_r[:])
                        rr.append(sb_r)
                    r_px1, r_py1, r_px2, r_py2, r_ap = rr

                    for mt in range(2):
                        g = gs[mt]
                        g_x1, g_y1 = g[:, 0:1], g[:, 1:2]
                        g_x2, g_y2 = g[:, 2:3], g[:, 3:4]
                        g_ag = g[:, 4:5]
                        g_bx1, g_by1 = g[:, 5:6], g[:, 6:7]
                        g_bx2, g_by2 = g[:, 7:8], g[:, 8:9]
                        g_sx, g_sy = g[:, 9:10], g[:, 10:11]
                        g_nby1, g_nby2 = g[:, 11:12], g[:, 12:13]
                        g_nbx1, g_nbx2 = g[:, 13:14], g[:, 14:15]

                        def gt_tile(nm, dt=F32, tg=None):
                            return grid.tile([P, CHUNK], dt, tag=tg or nm,
                                             name=f"{nm}_{i}_{ch}_{mt}")

                        # cost_class psum -> SBUF (ACT copy)
                        ps_cc = ps.tile([P, CHUNK], F32, tag="ps_cc",
                                        name=f"pscc_{i}_{ch}_{mt}")
                        nc.tensor.matmul(ps_cc[:], sb_oh[:, mt * P:(mt + 1) * P],
                                         gnegT[:, sl], start=True, stop=True)
                        cc_sb = gt_tile("cc_sb")
                        nc.scalar.copy(cc_sb[:], ps_cc[:])

                        wx = gt_tile("wx")
                        nc.vector._custom_dve(OP_WH, out=wx[:], in0=r_px2[:],
                                              in1=r_px1[:], s0=g_x2, s1=g_x1,
                                              imm2=0.0)
                        wy = gt_tile("wy")
                        nc.vector._custom_dve(OP_WH, out=wy[:], in0=r_py2[:],
                                              in1=r_py1[:], s0=g_y2, s1=g_y1,
                                              imm2=0.0)
                        inter = gt_tile("inter")
                        nc.gpsimd.tensor_mul(inter[:], wx[:], wy[:])
                        ewx = gt_tile("ewx", tg="wx")
                        nc.vector._custom_dve(OP_EWH, out=ewx[:], in0=r_px2[:],
                                              in1=r_px1[:], s0=g_x2, s1=g_x1,
                                              imm2=0.0)
                        ewy = gt_tile("ewy", tg="wy")
                        nc.vector._custom_dve(OP_EWH, out=ewy[:], in0=r_py2[:],
                                              in1=r_py1[:], s0=g_y2, s1=g_y1,
                                              imm2=0.0)
                        encl = gt_tile("encl")
                        nc.gpsimd.tensor_mul(encl[:], ewx[:], ewy[:])
                        union = gt_tile("union")
                        if union_dve:
                            nc.vector.scalar_tensor_tensor(
                                union[:], r_ap[:], g_ag, inter[:],
                                mybir.AluOpType.add, mybir.AluOpType.subtract)
                        else:
                            nc.gpsimd.tensor_scalar_add(union[:], r_ap[:], g_ag)
                            nc.gpsimd.tensor_sub(union[:], union[:], inter[:])
                        r_u = gt_tile("r_u")
                        nc.vector.reciprocal_approx_fast(out=r_u[:], in_=union[:])
                        iou2 = gt_tile("iou2")
                        nc.vector._custom_dve(OP_TTS, out=iou2[:], in0=inter[:],
                                              in1=r_u[:], s0=2.0, s1=0.0, imm2=0.0)
                        ue2 = grid.tile([P, CHUNK], F32, tag="ue2", bufs=3,
                            name=f"ue2_{i}_{ch}_{mt}")
                        if ue_act:
                            lnu = gt_tile("lnu", tg="r_e")
                            nc.scalar.activation(lnu[:], union[:], AF.Ln,
                                                 bias=b_zero[:])
                            lne = gt_tile("lne", tg="wy")
                            nc.scalar.activation(lne[:], encl[:], AF.Ln,
                                                 bias=b_zero[:])
                            dl = gt_tile("dl", tg="union")
                            nc.gpsimd.tensor_sub(dl[:], lnu[:], lne[:])
                            nc.scalar.activation(ue2[:], dl[:], AF.Exp,
                                                 bias=b_ln2[:])
                        else:
                            r_e = gt_tile("r_e")
                            nc.vector.reciprocal_approx_fast(out=r_e[:], in_=encl[:])
                            nc.vector._custom_dve(OP_TTS, out=ue2[:], in0=union[:],
                                                  in1=r_e[:], s0=2.0, s1=0.0,
                                                  imm2=0.0)
                        bbx = gt_tile("bbx")
                        if bbx_act:
                            ax1 = gt_tile("ax1", tg="inter2" if False else "encl")
                            nc.scalar.activation(ax1[:], r_px1[:], AF.Abs,
                                                 bias=g_nbx1)
                            ax2 = gt_tile("ax2", tg="union")
                            nc.scalar.activation(ax2[:], r_px2[:], AF.Abs,
                                                 bias=g_nbx2)
                            nc.gpsimd.tensor_add(bbx[:], ax1[:], ax2[:])
                        else:
                            nc.vector._custom_dve(OP_BBOX2, out=bbx[:],
                                                  in0=r_px1[:], in1=r_px2[:],
                                                  s0=g_bx1, s1=g_bx2, imm2=0.0)
                        bby = gt_tile("bby")
                        if bby_act:
                            ab1 = gt_tile("ab1", tg="wx")
                            nc.scalar.activation(ab1[:], r_py1[:], AF.Abs,
                                                 bias=g_nby1, scale=g_sy)
                            ab2 = gt_tile("ab2", tg="wy")
                            nc.scalar.activation(ab2[:], r_py2[:], AF.Abs,
                                                 bias=g_nby2, scale=g_sy)
                            nc.gpsimd.tensor_add(bby[:], ab1[:], ab2[:])
                        else:
                            nc.vector._custom_dve(OP_BBOX2, out=bby[:],
                                                  in0=r_py1[:], in1=r_py2[:],
                                                  s0=g_by1, s1=g_by2, imm2=0.0)
                        # merges on GPSIMD (in-place where safe)
                        nc.gpsimd.tensor_add(ue2[:], ue2[:], iou2[:])      # m1
                        if sxy_act:
                            nc.scalar.activation(bbx[:], bbx[:], AF.Copy,
                                                 scale=g_sx)
                        else:
                            nc.gpsimd.tensor_scalar_mul(bbx[:], bbx[:], g_sx)
                        if not bby_act:
                            nc.gpsimd.tensor_scalar_mul(bby[:], bby[:], g_sy)
                        nc.gpsimd.tensor_add(bbx[:], bbx[:], bby[:])       # m2
                        nc.gpsimd.tensor_sub(ue2[:], ue2[:], bbx[:])       # m3
                        hf, hsl = ch // 4, slice((ch % 4) * CHUNK,
                                                 (ch % 4 + 1) * CHUNK)
                        nc.gpsimd.tensor_add(t_neg[mt][hf][:, hsl], ue2[:],
                                             cc_sb[:])

                        csl = slice(ch * 8, (ch + 1) * 8)
                        nc.vector.max(out=t_icv[mt][:, csl], in_=iou2[:])

                for mt in range(2):
                    nc.scalar.dma_start(negc[i, mt, :, 0:N // 2],
                                        t_neg[mt][0][:])
                    for q in range(2):
                        nc.sync.dma_start(
                            negc[i, mt, :, N // 2 + q * 1024:
                                 N // 2 + (q + 1) * 1024],
                            t_neg[mt][1][:, q * 1024:(q + 1) * 1024])
                    nc.gpsimd.dma_start(icv[i, mt][:], t_icv[mt][:])

    nc.compile()
    return nc


BEST_OPTS = {"bby_act": True}


def _get_nc():
    global _NC_CACHE
    if _NC_CACHE is None:
        _NC_CACHE = _build_nc(BEST_OPTS)
    return _NC_CACHE


# ---------------- host side ------------------------------------------------


def _pack_inputs(pred_logits, pred_boxes, boxes_xyxy, labels,
                 image_size_xyxy, image_size_xyxy_tgt):
    f32 = np.float32
    pred_logits = np.asarray(pred_logits, f32)
    pred_boxes = np.asarray(pred_boxes, f32)
    boxes_xyxy = np.asarray(boxes_xyxy, f32)
    labels = np.asarray(labels)
    img = np.asarray(image_size_xyxy, f32)
    img_t = np.asarray(image_size_xyxy_tgt, f32)

    # lg: [B, 128, 32*C] packed so SBUF partition p holds anchors {t*128+p}
    lg = np.ascontiguousarray(
        pred_logits.reshape(B, 32, P, C).transpose(0, 2, 1, 3)
    ).reshape(B, P, 32 * C)

    px1, py1, px2, py2 = [pred_boxes[:, :, j] for j in range(4)]
    ap = (px2 - px1) * (py2 - py1)
    const4 = np.full((B, N), -4.0, f32)
    prow = np.stack([px1, py1, px2, py2, ap, const4], axis=1)

    gx1, gy1, gx2, gy2 = [boxes_xyxy[:, :, j] for j in range(4)]
    ag = (gx2 - gx1) * (gy2 - gy1)
    # bbox scaling: 5*|px/w - gx/wt| = (5/w)*|px - gx*(w/wt)|
    w, h = img[:, 0:1], img[:, 1:2]
    wt, ht = img_t[:, 0:1], img_t[:, 1:2]
    sx = np.broadcast_to(5.0 / w, (B, M)).astype(f32)
    sy = np.broadcast_to(5.0 / h, (B, M)).astype(f32)
    gcol = np.stack([gx1, gy1, gx2, gy2, ag,
                     gx1 * (w / wt), gy1 * (h / ht),
                     gx2 * (w / wt), gy2 * (h / ht), sx, sy,
                     -5.0 * gy1 / ht, -5.0 * gy2 / ht,
                     -gx1 * (w / wt), -gx2 * (w / wt)],
                    axis=2).astype(f32)          # [B, M, 15]
    gcol = gcol.reshape(B, 2, P, 15)

    ohot = np.zeros((B, 81, M), f32)
    ohot[:, 80, :] = 1.0
    bidx = np.repeat(np.arange(B), M)
    ohot[bidx, labels.astype(np.int64).ravel(), np.tile(np.arange(M), B)] = 1.0

    emat = np.zeros((KROWS, KROWS * P), f32)
    for r in range(KROWS):
        emat[r, r * P:(r + 1) * P] = 1.0

    in_maps = []
    for c in range(NCORES):
        s = slice(IPC * c, IPC * (c + 1))
        in_maps.append({
            "lg": np.ascontiguousarray(lg[s]),
            "prow": np.ascontiguousarray(prow[s].astype(f32)),
            "gcol": np.ascontiguousarray(gcol[s]),
            "oh": np.ascontiguousarray(ohot[s]),
            "emat": emat,
        })
    return in_maps


def _assemble(negcT, icv):
    """negcT [B, M, N] f32 (= -cost^T); icv [B, M, NCAND] f32 (~2*iou)."""
    f32 = np.float32
    R = B * M

    # dyn_k from iou candidates (values are 2*iou)
    iv = -np.sort(-icv.reshape(R, NCAND), axis=1)[:, :OTA_K] * f32(0.5)
    dyn_k = np.maximum(iv.sum(1).astype(np.int32), 1)            # [R]

    # exact top-10 smallest cost (= largest negcost), ties -> lowest index
    flat = negcT.reshape(R, N)
    part = np.argpartition(-flat, OTA_K + 2, axis=1)[:, :OTA_K + 2]
    pv = np.take_along_axis(flat, part, 1)
    order = np.lexsort((part, -pv), axis=-1)[:, :OTA_K]
    idx10 = np.take_along_axis(part, order, 1)                   # [R, 10]

    keep = np.arange(OTA_K)[None, :] < dyn_k[:, None]            # [R, 10]
    mm = np.zeros((B, N, M), bool)
    bb = np.broadcast_to(np.repeat(np.arange(B), M)[:, None], (R, OTA_K))
    mcol = np.broadcast_to(np.tile(np.arange(M), B)[:, None], (R, OTA_K))
    mm[bb[keep], idx10[keep], mcol[keep]] = True

    s = mm.sum(2)
    multi = s > 1
    rm = np.argmax(negcT, axis=1)            # [B, N] argmin-cost gt per anchor
    bbn, nn = np.nonzero(multi)
    mm[bbn, nn, :] = False
    mm[bbn, nn, rm[bbn, nn]] = True

    u = mm.sum(1)
    un = u == 0                              # [B, M]
    cm = np.argmax(negcT, axis=2)            # [B, M] argmin-cost anchor per gt
    bbm, mmi = np.nonzero(un)
    mm[bbm, cm[bbm, mmi], mmi] = True

    sel = mm.any(2)
    gt_idx = np.argmax(mm, axis=2).astype(np.int32)
    cost = (-negcT).transpose(0, 2, 1)
    return sel, gt_idx, cost


def kernel(pred_logits, pred_boxes, boxes_xyxy, labels,
           image_size_xyxy, image_size_xyxy_tgt):
    nc = _get_nc()
    in_maps = _pack_inputs(pred_logits, pred_boxes, boxes_xyxy, labels,
                           image_size_xyxy, image_size_xyxy_tgt)
    res = run_bass_kernel_spmd(nc, in_maps, list(range(NCORES)))
    outs = res.results

    negcT = np.empty((B, M, N), np.float32)
    icv = np.empty((B, M, NCAND), np.float32)
    for c in range(NCORES):
        o = outs[c]
        for i in range(IPC):
            b = IPC * c + i
            negcT[b] = o["negc"][i].reshape(M, N)
            icv[b] = o["icv"][i].reshape(M, NCAND)

    return _assemble(negcT, icv)
